# revision 1
# baseline (speedup 1.0000x reference)
"""Bass/Tile TRN2 kernel for a 3x3 locally-connected (unshared-weight) layer.

Computation (per batch row b, grid unit h, hw = 256*256):
    y[b,h] = sigmoid( sum_o x[b, nbr_idx[o,h]] * (valid[o,h] ? weights[o,h] : 0) )
    y[b,h] = sigmoid(0) = 0.5 where ~fault_mask[h] (mask applied pre-sigmoid)

Strategy: the neighbor gather is a fixed local stencil (verified on host at
call time).  With x transposed to (hw, batch), the layer is a block-banded
matmul: for output chunks of C=126 units, each dy-band's input window is a
128-row slice of x_t, and the per-chunk weight block is a (128, 128)
tridiagonal-ish matrix.  TensorE accumulates 4 window-blocks per chunk
(3x K=128 dy-bands + 1x K=8 merged edge block) into PSUM; four chunks share
a two-bank PSUM tile, and ScalarE applies sigmoid per quad.  Faulted units
output the constant sigmoid(0)=0.5, filled in on the host during unshard.

Sharding: hw is split 8 ways (66 chunks of 126 units per core, padded grid of
528 chunks); batch (256) rides along the matmul free dimension.  Every core
runs an identical program; boundary effects are encoded in host-built
zero-padded windows / zero weight blocks.
"""

import numpy as np
import ml_dtypes

BATCH = 256
HW = 65536
N_CONN = 9
C = 126               # output chunk size (so a dy-band window is C+2=128 rows)
NCHUNK_PAD = 528      # padded global chunk count, divisible by 8
NCORES = 8
CPC = NCHUNK_PAD // NCORES   # 66 chunks per core
NWIN = 72                    # window slots per core (locals j .. j+4 used)
PAD = 512                    # zero-row padding on each end of x_t
GRID = NCHUNK_PAD * C        # 66528 padded grid extent
SLAB = 6                     # chunks per weight-slab DMA
NSLAB = CPC // SLAB          # 11
XWSLAB = 4                   # windows per resident xw tile

_BF16 = ml_dtypes.bfloat16


def _build_blocks(weights, nbr_idx, valid):
    """Scatter effective weights into per-chunk matmul blocks.

    Returns (WM, WE) float32 (weight-block column dim padded 126 -> 128 so
    every lhsT has exactly 128 columns, enabling fast weight load):
      WM: (NCHUNK_PAD, 128, 384)  main blocks, free layout [dy0 | dy+1 | dy-1]
      WE: (NCHUNK_PAD, 8, 128)    merged edge blocks (rows 0:4 dy+1, 4:8 dy-1)

    For chunk J (outputs h in [126J, 126J+126)), the 4 pieces read x_t rows:
      P1 main dy0 : window J   rows [126J-1,    126J+127)
      P2 main dy+1: window J+2 rows [126J+251,  126J+379)
      P3 main dy-1: window J-2 rows [126J-253,  126J-125)
      P4 edge rows 0:4  [126J+379, 126J+383),  rows 4:8 [126J-257, 126J-253)
    Raises ValueError if some valid (o,h) connection is not coverable.
    """
    h = np.arange(HW, dtype=np.int64)
    J = h // C
    p = h % C
    g = nbr_idx.astype(np.int64)
    vm = valid.astype(bool)
    w_eff = np.where(vm, weights.astype(np.float32), 0.0)

    Jb = np.broadcast_to(J, g.shape)
    pb = np.broadcast_to(p, g.shape)

    r1 = g - (C * Jb - 1)
    r2 = g - (C * (Jb + 2) - 1)
    r3 = g - (C * (Jb - 2) - 1)
    r4 = g - (C * Jb + 379)            # edge dy+1 -> rows 0:4
    r5 = g - (C * Jb - 257) + 4        # edge dy-1 -> rows 4:8

    in1 = (r1 >= 0) & (r1 < 128)
    in2 = (r2 >= 0) & (r2 < 128)
    in3 = (r3 >= 0) & (r3 < 128)
    in4 = (r4 >= 0) & (r4 < 4)
    in5 = (r5 >= 4) & (r5 < 8)

    m1 = vm & in1
    m2 = vm & in2 & ~m1
    m3 = vm & in3 & ~m1 & ~m2
    m4 = vm & in4 & ~m1 & ~m2 & ~m3
    m5 = vm & in5 & ~m1 & ~m2 & ~m3 & ~m4
    covered = m1 | m2 | m3 | m4 | m5
    if not np.all(covered | ~vm):
        raise ValueError(
            "nbr_idx is not coverable by the local-stencil kernel "
            f"({np.count_nonzero(vm & ~covered)} uncovered connections)"
        )

    WM = np.zeros((NCHUNK_PAD, 128, 384), dtype=np.float32)
    WE = np.zeros((NCHUNK_PAD, 8, 128), dtype=np.float32)
    for m, r, arr, coff in (
        (m1, r1, WM, 0),
        (m2, r2, WM, 128),
        (m3, r3, WM, 256),
        (m4, r4, WE, 0),
        (m5, r5, WE, 0),
    ):
        np.add.at(arr, (Jb[m], r[m], coff + pb[m]), w_eff[m])
    return WM, WE


def _build_program():
    import concourse.bacc as bacc
    import concourse.mybir as mybir
    from concourse import tile
    from concourse._compat import axon_active

    nc = bacc.Bacc(
        "TRN2",
        target_bir_lowering=False,
        debug=not axon_active(),
        num_devices=NCORES,
    )
    f32 = mybir.dt.float32
    bf16 = mybir.dt.bfloat16

    xw_d = nc.dram_tensor("xw", [128, NWIN * 256], bf16, kind="ExternalInput")
    wm_d = nc.dram_tensor("wm", [NSLAB, 128, SLAB * 384], bf16, kind="ExternalInput")
    xce_d = nc.dram_tensor("xce", [NSLAB, 8, (SLAB // 2) * 256], bf16, kind="ExternalInput")
    xco_d = nc.dram_tensor("xco", [NSLAB, 8, (SLAB // 2) * 256], bf16, kind="ExternalInput")
    wee_d = nc.dram_tensor("wee", [NSLAB, 8, (SLAB // 2) * 128], bf16, kind="ExternalInput")
    weo_d = nc.dram_tensor("weo", [NSLAB, 8, (SLAB // 2) * 128], bf16, kind="ExternalInput")
    yt_d = nc.dram_tensor("yt", [C, CPC * 256], f32, kind="ExternalOutput")

    with tile.TileContext(nc) as tc:
        with (
            tc.tile_pool(name="xw", bufs=1) as xw_pool,
            tc.tile_pool(name="const", bufs=1) as const_pool,
            tc.tile_pool(name="wm", bufs=4) as wm_pool,
            tc.tile_pool(name="xc", bufs=3) as xc_pool,
            tc.tile_pool(name="we", bufs=3) as we_pool,
            tc.tile_pool(name="out", bufs=6) as out_pool,
            tc.tile_pool(name="psum", bufs=4, space="PSUM") as psum_pool,
        ):
            # resident x windows. Two small 4-window tiles first (fast
            # start), then 8-window tiles (4KB DMA packets), issued in
            # consumption order.
            xw_sizes = [4, 4] + [8] * 8
            xw_base = [sum(xw_sizes[:i]) for i in range(len(xw_sizes))]
            xw_tiles = [
                xw_pool.tile([128, n * 256], bf16, tag=f"xw{s}", name=f"xw{s}")
                for s, n in enumerate(xw_sizes)
            ]
            for s, n in enumerate(xw_sizes):
                nc.gpsimd.dma_start(
                    out=xw_tiles[s][:, :],
                    in_=xw_d[:, xw_base[s] * 256 : (xw_base[s] + n) * 256],
                )

            # PE pre-warm: dummy matmuls on zeroed SBUF while the first input
            # DMAs are in flight, so the HAM clock-gate opens (1.2 -> 2.4 GHz)
            # before the real matmul stream begins.
            warm_sb = const_pool.tile([128, 640], bf16, tag="warm")
            nc.vector.memset(warm_sb[:, :], 0.0)
            warm_ps = psum_pool.tile([128, 1024], f32, tag="ps")
            for _ in range(10):
                nc.tensor.matmul(
                    warm_ps[:, 0:512],
                    warm_sb[:, 0:128],
                    warm_sb[:, 128:640],
                    start=True,
                    stop=True,
                )

            def win(w):  # rhs AP for local window index w (full 128 rows)
                ti = 0
                while w >= xw_base[ti] + xw_sizes[ti]:
                    ti += 1
                o = w - xw_base[ti]
                return xw_tiles[ti][:, o * 256 : (o + 1) * 256]

            for s in range(NSLAB):
                wm_sb = wm_pool.tile([128, SLAB * 384], bf16)
                nc.sync.dma_start(out=wm_sb[:, :], in_=wm_d[s])
                xc_sb = xc_pool.tile([8, (SLAB // 2) * 256], bf16)
                nc.sync.dma_start(out=xc_sb[:, :], in_=xce_d[s])
                xco_sb = xc_pool.tile([8, (SLAB // 2) * 256], bf16, tag="xco")
                nc.sync.dma_start(out=xco_sb[:, :], in_=xco_d[s])
                we_sb = we_pool.tile([8, (SLAB // 2) * 128], bf16)
                nc.sync.dma_start(out=we_sb[:, :], in_=wee_d[s])
                weo_sb = we_pool.tile([8, (SLAB // 2) * 128], bf16, tag="weo")
                nc.sync.dma_start(out=weo_sb[:, :], in_=weo_d[s])

                for q2 in range(SLAB // 2):  # chunk pairs within slab
                    pi = s * (SLAB // 2) + q2  # global pair index
                    if pi % 2 == 0:
                        ps = psum_pool.tile([128, 1024], f32)
                    # start=True only on the pair's first MM: it clears the
                    # has_written bits of this pair's whole PSUM bank; every
                    # later MM (start=False) overwrites fresh cells and
                    # accumulates onto written ones, so MM order is free.
                    for half in range(2):
                        q = q2 * 2 + half
                        j = s * SLAB + q
                        co = (pi % 2) * 512 + half * 256
                        w0 = q * 384
                        nc.tensor.matmul(
                            ps[:, co : co + 256],
                            wm_sb[:, w0 : w0 + 128],
                            win(j + 2),
                            start=(half == 0),
                            stop=False,
                            skip_group_check=True,
                        )
                        nc.tensor.matmul(
                            ps[:, co : co + 256],
                            wm_sb[:, w0 + 128 : w0 + 256],
                            win(j + 4),
                            start=False,
                            stop=False,
                            skip_group_check=True,
                        )
                        nc.tensor.matmul(
                            ps[:, co : co + 256],
                            wm_sb[:, w0 + 256 : w0 + 384],
                            win(j),
                            start=False,
                            stop=False,
                            skip_group_check=True,
                        )
                    # packed edge MMs: even chunk in array row group 0,
                    # odd chunk in row group 1 -- they execute concurrently.
                    co = (pi % 2) * 512
                    nc.tensor.matmul(
                        ps[:, co : co + 256],
                        we_sb[0:8, q2 * 128 : (q2 + 1) * 128],
                        xc_sb[0:8, q2 * 256 : (q2 + 1) * 256],
                        start=False,
                        stop=False,
                        skip_group_check=True,
                    )
                    nc.tensor.matmul(
                        ps[:, co + 256 : co + 512],
                        weo_sb[0:8, q2 * 128 : (q2 + 1) * 128],
                        xco_sb[0:8, q2 * 256 : (q2 + 1) * 256],
                        start=False,
                        stop=True,
                        skip_group_check=True,
                    )

                    npair = NSLAB * (SLAB // 2)
                    if s == NSLAB - 1:
                        # tail: per-pair sigmoid+store so the final store is
                        # small and starts right after the last matmuls
                        ot = out_pool.tile([128, 1024], f32)
                        co = (pi % 2) * 512
                        nc.scalar.activation(
                            ot[0:C, 0:512],
                            ps[0:C, co : co + 512],
                            mybir.ActivationFunctionType.Sigmoid,
                            bias=0.0,
                            scale=1.0,
                        )
                        nc.gpsimd.dma_start(
                            out=yt_d[:, pi * 512 : (pi + 1) * 512],
                            in_=ot[0:C, 0:512],
                        )
                    elif pi % 2 == 1 or pi == npair - 1:
                        width = (pi % 2 + 1) * 512
                        ot = out_pool.tile([128, 1024], f32)
                        nc.scalar.activation(
                            ot[0:C, 0:width],
                            ps[0:C, 0:width],
                            mybir.ActivationFunctionType.Sigmoid,
                            bias=0.0,
                            scale=1.0,
                        )
                        j0 = (pi // 2) * 4  # first chunk of this store group
                        nc.gpsimd.dma_start(
                            out=yt_d[:, j0 * 256 : j0 * 256 + width],
                            in_=ot[0:C, 0:width],
                        )
    nc.compile()
    return nc


TRACE = False          # set by test harness to capture an NTFF profile
LAST_RESULTS = None    # BassKernelResults of the most recent run
_NC_CACHE = None       # compiled program, reused across calls


def kernel(x, weights, nbr_idx, valid, fault_mask):
    global LAST_RESULTS
    from concourse.bass_utils import run_bass_kernel_spmd

    x = np.asarray(x)
    out_dtype = x.dtype

    WM, WE = _build_blocks(np.asarray(weights), np.asarray(nbr_idx), np.asarray(valid))
    WM = WM.astype(_BF16)
    WE = WE.astype(_BF16)

    # x transposed to (hw, batch), zero-padded, bf16
    xt_pad = np.zeros((PAD + GRID + PAD, BATCH), dtype=_BF16)
    xt_pad[PAD : PAD + HW] = np.ascontiguousarray(x.T).astype(_BF16)

    k128 = np.arange(128)
    in_maps = []
    for c in range(NCORES):
        j0 = c * CPC
        # main windows: global window t in [j0-2, j0+70), rows PAD + 126*t - 1 + k
        tglob = j0 - 2 + np.arange(NWIN)
        rows = (PAD + C * tglob - 1)[:, None] + k128[None, :]  # (72, 128)
        xw = np.ascontiguousarray(xt_pad[rows].transpose(1, 0, 2))  # (128, 72, 256)

        # merged edge windows: rows 0:4 from [126J+379, +4), 4:8 from [126J-257, +4)
        Jc = j0 + np.arange(CPC)
        k4 = np.arange(4)
        erows_p = (PAD + C * Jc + 379)[:, None] + k4[None, :]  # (66, 4)
        erows_m = (PAD + C * Jc - 257)[:, None] + k4[None, :]  # (66, 4)
        xc = np.concatenate(
            [xt_pad[erows_p], xt_pad[erows_m]], axis=1
        )  # (66, 8, 256)
        xce = xc[0::2].reshape(NSLAB, SLAB // 2, 8, 256).transpose(0, 2, 1, 3)
        xco = xc[1::2].reshape(NSLAB, SLAB // 2, 8, 256).transpose(0, 2, 1, 3)

        wm_c = (
            WM[j0 : j0 + CPC]
            .reshape(NSLAB, SLAB, 128, 384)
            .transpose(0, 2, 1, 3)
        )  # (11, 128, 6, 384)
        we_cc = WE[j0 : j0 + CPC]
        wee = we_cc[0::2].reshape(NSLAB, SLAB // 2, 8, 128).transpose(0, 2, 1, 3)
        weo = we_cc[1::2].reshape(NSLAB, SLAB // 2, 8, 128).transpose(0, 2, 1, 3)

        in_maps.append(
            {
                "xw": xw.reshape(128, NWIN * 256),
                "wm": np.ascontiguousarray(wm_c).reshape(NSLAB, 128, SLAB * 384),
                "xce": np.ascontiguousarray(xce).reshape(NSLAB, 8, (SLAB // 2) * 256),
                "xco": np.ascontiguousarray(xco).reshape(NSLAB, 8, (SLAB // 2) * 256),
                "wee": np.ascontiguousarray(wee).reshape(NSLAB, 8, (SLAB // 2) * 128),
                "weo": np.ascontiguousarray(weo).reshape(NSLAB, 8, (SLAB // 2) * 128),
            }
        )

    global _NC_CACHE
    if _NC_CACHE is None:
        _NC_CACHE = _build_program()
    nc = _NC_CACHE
    res = run_bass_kernel_spmd(
        nc, in_maps, core_ids=list(range(NCORES)), trace=TRACE
    )
    LAST_RESULTS = res

    # unshard: per-core yt is (126, 66*256) partition-major -> (B, HW)
    yts = [
        r["yt"].reshape(C, CPC, BATCH).transpose(1, 0, 2).reshape(CPC * C, BATCH)
        for r in res.results
    ]
    yt = np.concatenate(yts, axis=0)  # (66528, 256)
    y = np.ascontiguousarray(yt[:HW].T).astype(out_dtype, copy=False)
    # faulted units: reference computes sigmoid(where(fault, y, 0)) -> 0.5
    fault = np.asarray(fault_mask).astype(bool)
    y[:, ~fault] = np.float32(0.5)
    return y



# revision 3
# speedup vs baseline: 1.0396x; 1.0396x over previous
"""Bass/Tile TRN2 kernel for a 3x3 locally-connected (unshared-weight) layer.

Computation (per batch row b, grid unit h, hw = 256*256):
    y[b,h] = sigmoid( sum_o x[b, nbr_idx[o,h]] * (valid[o,h] ? weights[o,h] : 0) )
    y[b,h] = sigmoid(0) = 0.5 where ~fault_mask[h] (mask applied pre-sigmoid)

Strategy: the neighbor gather is a fixed local stencil (verified on host at
call time).  With x transposed to (hw, batch), the layer is a block-banded
matmul: for output chunks of C=126 units, each dy-band's input window is a
128-row slice of x_t, and the per-chunk weight block is a (128, 128)
tridiagonal-ish matrix.  TensorE accumulates 4 window-blocks per chunk
(3x K=128 dy-bands + 1x K=8 merged edge block) into PSUM; four chunks share
a two-bank PSUM tile, and ScalarE applies sigmoid per quad.  Faulted units
output the constant sigmoid(0)=0.5, filled in on the host during unshard.

Sharding: hw is split 8 ways (66 chunks of 126 units per core, padded grid of
528 chunks); batch (256) rides along the matmul free dimension.  Every core
runs an identical program; boundary effects are encoded in host-built
zero-padded windows / zero weight blocks.
"""

import numpy as np
import ml_dtypes

BATCH = 256
HW = 65536
N_CONN = 9
C = 126               # output chunk size (so a dy-band window is C+2=128 rows)
NCHUNK_PAD = 528      # padded global chunk count, divisible by 8
NCORES = 8
CPC = NCHUNK_PAD // NCORES   # 66 chunks per core
NWIN = 72                    # window slots per core (locals j .. j+4 used)
PAD = 512                    # zero-row padding on each end of x_t
GRID = NCHUNK_PAD * C        # 66528 padded grid extent
SLAB = 6                     # chunks per weight-slab DMA
NSLAB = CPC // SLAB          # 11
XWSLAB = 4                   # windows per resident xw tile

_BF16 = ml_dtypes.bfloat16


def _build_blocks(weights, nbr_idx, valid):
    """Scatter effective weights into per-chunk matmul blocks.

    Returns (WM, WE) float32 (weight-block column dim padded 126 -> 128 so
    every lhsT has exactly 128 columns, enabling fast weight load):
      WM: (NCHUNK_PAD, 128, 384)  main blocks, free layout [dy0 | dy+1 | dy-1]
      WE: (NCHUNK_PAD, 8, 128)    merged edge blocks (rows 0:4 dy+1, 4:8 dy-1)

    For chunk J (outputs h in [126J, 126J+126)), the 4 pieces read x_t rows:
      P1 main dy0 : window J   rows [126J-1,    126J+127)
      P2 main dy+1: window J+2 rows [126J+251,  126J+379)
      P3 main dy-1: window J-2 rows [126J-253,  126J-125)
      P4 edge rows 0:4  [126J+379, 126J+383),  rows 4:8 [126J-257, 126J-253)
    Raises ValueError if some valid (o,h) connection is not coverable.
    """
    h = np.arange(HW, dtype=np.int64)
    J = h // C
    p = h % C
    g = nbr_idx.astype(np.int64)
    vm = valid.astype(bool)
    w_eff = np.where(vm, weights.astype(np.float32), 0.0)

    Jb = np.broadcast_to(J, g.shape)
    pb = np.broadcast_to(p, g.shape)

    r1 = g - (C * Jb - 1)
    r2 = g - (C * (Jb + 2) - 1)
    r3 = g - (C * (Jb - 2) - 1)
    r4 = g - (C * Jb + 379)            # edge dy+1 -> rows 0:4
    r5 = g - (C * Jb - 257) + 4        # edge dy-1 -> rows 4:8

    in1 = (r1 >= 0) & (r1 < 128)
    in2 = (r2 >= 0) & (r2 < 128)
    in3 = (r3 >= 0) & (r3 < 128)
    in4 = (r4 >= 0) & (r4 < 4)
    in5 = (r5 >= 4) & (r5 < 8)

    m1 = vm & in1
    m2 = vm & in2 & ~m1
    m3 = vm & in3 & ~m1 & ~m2
    m4 = vm & in4 & ~m1 & ~m2 & ~m3
    m5 = vm & in5 & ~m1 & ~m2 & ~m3 & ~m4
    covered = m1 | m2 | m3 | m4 | m5
    if not np.all(covered | ~vm):
        raise ValueError(
            "nbr_idx is not coverable by the local-stencil kernel "
            f"({np.count_nonzero(vm & ~covered)} uncovered connections)"
        )

    WM = np.zeros((NCHUNK_PAD, 128, 384), dtype=np.float32)
    WE = np.zeros((NCHUNK_PAD, 8, 128), dtype=np.float32)
    for m, r, arr, coff in (
        (m1, r1, WM, 0),
        (m2, r2, WM, 128),
        (m3, r3, WM, 256),
        (m4, r4, WE, 0),
        (m5, r5, WE, 0),
    ):
        np.add.at(arr, (Jb[m], r[m], coff + pb[m]), w_eff[m])
    return WM, WE


def _build_program():
    import concourse.bacc as bacc
    import concourse.mybir as mybir
    from concourse import tile
    from concourse._compat import axon_active

    nc = bacc.Bacc(
        "TRN2",
        target_bir_lowering=False,
        debug=not axon_active(),
        num_devices=NCORES,
    )
    f32 = mybir.dt.float32
    bf16 = mybir.dt.bfloat16

    xw_d = nc.dram_tensor("xw", [128, NWIN * 256], bf16, kind="ExternalInput")
    wm_d = nc.dram_tensor("wm", [NSLAB, 128, SLAB * 384], bf16, kind="ExternalInput")
    xce_d = nc.dram_tensor("xce", [NSLAB, 8, (SLAB // 2) * 256], bf16, kind="ExternalInput")
    xco_d = nc.dram_tensor("xco", [NSLAB, 8, (SLAB // 2) * 256], bf16, kind="ExternalInput")
    wee_d = nc.dram_tensor("wee", [NSLAB, 8, (SLAB // 2) * 128], bf16, kind="ExternalInput")
    weo_d = nc.dram_tensor("weo", [NSLAB, 8, (SLAB // 2) * 128], bf16, kind="ExternalInput")
    yt_d = nc.dram_tensor("yt", [C, CPC * 256], bf16, kind="ExternalOutput")

    with tile.TileContext(nc) as tc:
        with (
            tc.tile_pool(name="xw", bufs=1) as xw_pool,
            tc.tile_pool(name="const", bufs=1) as const_pool,
            tc.tile_pool(name="wm", bufs=4) as wm_pool,
            tc.tile_pool(name="xc", bufs=3) as xc_pool,
            tc.tile_pool(name="we", bufs=3) as we_pool,
            tc.tile_pool(name="out", bufs=6) as out_pool,
            tc.tile_pool(name="psum", bufs=4, space="PSUM") as psum_pool,
        ):
            # resident x windows. Two small 4-window tiles first (fast
            # start), then 8-window tiles (4KB DMA packets), issued in
            # consumption order.
            xw_sizes = [4, 4] + [8] * 8
            xw_base = [sum(xw_sizes[:i]) for i in range(len(xw_sizes))]
            xw_tiles = [
                xw_pool.tile([128, n * 256], bf16, tag=f"xw{s}", name=f"xw{s}")
                for s, n in enumerate(xw_sizes)
            ]
            for s, n in enumerate(xw_sizes):
                nc.gpsimd.dma_start(
                    out=xw_tiles[s][:, :],
                    in_=xw_d[:, xw_base[s] * 256 : (xw_base[s] + n) * 256],
                )

            # PE pre-warm: dummy matmuls on zeroed SBUF while the first input
            # DMAs are in flight, so the HAM clock-gate opens (1.2 -> 2.4 GHz)
            # before the real matmul stream begins.
            warm_sb = const_pool.tile([128, 640], bf16, tag="warm")
            nc.vector.memset(warm_sb[:, :], 0.0)
            warm_ps = psum_pool.tile([128, 1024], f32, tag="ps")
            for _ in range(10):
                nc.tensor.matmul(
                    warm_ps[:, 0:512],
                    warm_sb[:, 0:128],
                    warm_sb[:, 128:640],
                    start=True,
                    stop=True,
                )

            def win(w):  # rhs AP for local window index w (full 128 rows)
                ti = 0
                while w >= xw_base[ti] + xw_sizes[ti]:
                    ti += 1
                o = w - xw_base[ti]
                return xw_tiles[ti][:, o * 256 : (o + 1) * 256]

            for s in range(NSLAB):
                wm_sb = wm_pool.tile([128, SLAB * 384], bf16)
                nc.sync.dma_start(out=wm_sb[:, :], in_=wm_d[s])
                xc_sb = xc_pool.tile([8, (SLAB // 2) * 256], bf16)
                nc.sync.dma_start(out=xc_sb[:, :], in_=xce_d[s])
                xco_sb = xc_pool.tile([8, (SLAB // 2) * 256], bf16, tag="xco")
                nc.sync.dma_start(out=xco_sb[:, :], in_=xco_d[s])
                we_sb = we_pool.tile([8, (SLAB // 2) * 128], bf16)
                nc.sync.dma_start(out=we_sb[:, :], in_=wee_d[s])
                weo_sb = we_pool.tile([8, (SLAB // 2) * 128], bf16, tag="weo")
                nc.sync.dma_start(out=weo_sb[:, :], in_=weo_d[s])

                for q2 in range(SLAB // 2):  # chunk pairs within slab
                    pi = s * (SLAB // 2) + q2  # global pair index
                    if pi % 2 == 0:
                        ps = psum_pool.tile([128, 1024], f32)
                    # start=True only on the pair's first MM: it clears the
                    # has_written bits of this pair's whole PSUM bank; every
                    # later MM (start=False) overwrites fresh cells and
                    # accumulates onto written ones, so MM order is free.
                    for half in range(2):
                        q = q2 * 2 + half
                        j = s * SLAB + q
                        co = (pi % 2) * 512 + half * 256
                        w0 = q * 384
                        nc.tensor.matmul(
                            ps[:, co : co + 256],
                            wm_sb[:, w0 : w0 + 128],
                            win(j + 2),
                            start=(half == 0),
                            stop=False,
                            skip_group_check=True,
                        )
                        nc.tensor.matmul(
                            ps[:, co : co + 256],
                            wm_sb[:, w0 + 128 : w0 + 256],
                            win(j + 4),
                            start=False,
                            stop=False,
                            skip_group_check=True,
                        )
                        nc.tensor.matmul(
                            ps[:, co : co + 256],
                            wm_sb[:, w0 + 256 : w0 + 384],
                            win(j),
                            start=False,
                            stop=False,
                            skip_group_check=True,
                        )
                    # packed edge MMs: even chunk in array row group 0,
                    # odd chunk in row group 1 -- they execute concurrently.
                    co = (pi % 2) * 512
                    nc.tensor.matmul(
                        ps[:, co : co + 256],
                        we_sb[0:8, q2 * 128 : (q2 + 1) * 128],
                        xc_sb[0:8, q2 * 256 : (q2 + 1) * 256],
                        start=False,
                        stop=False,
                        skip_group_check=True,
                    )
                    nc.tensor.matmul(
                        ps[:, co + 256 : co + 512],
                        weo_sb[0:8, q2 * 128 : (q2 + 1) * 128],
                        xco_sb[0:8, q2 * 256 : (q2 + 1) * 256],
                        start=False,
                        stop=True,
                        skip_group_check=True,
                    )

                    npair = NSLAB * (SLAB // 2)
                    if s == NSLAB - 1:
                        # tail: per-pair sigmoid+store so the final store is
                        # small and starts right after the last matmuls
                        ot = out_pool.tile([128, 1024], bf16)
                        co = (pi % 2) * 512
                        nc.scalar.activation(
                            ot[0:C, 0:512],
                            ps[0:C, co : co + 512],
                            mybir.ActivationFunctionType.Sigmoid,
                            bias=0.0,
                            scale=1.0,
                        )
                        nc.gpsimd.dma_start(
                            out=yt_d[:, pi * 512 : (pi + 1) * 512],
                            in_=ot[0:C, 0:512],
                        )
                    elif pi % 2 == 1 or pi == npair - 1:
                        width = (pi % 2 + 1) * 512
                        ot = out_pool.tile([128, 1024], bf16)
                        nc.scalar.activation(
                            ot[0:C, 0:width],
                            ps[0:C, 0:width],
                            mybir.ActivationFunctionType.Sigmoid,
                            bias=0.0,
                            scale=1.0,
                        )
                        j0 = (pi // 2) * 4  # first chunk of this store group
                        nc.gpsimd.dma_start(
                            out=yt_d[:, j0 * 256 : j0 * 256 + width],
                            in_=ot[0:C, 0:width],
                        )
    nc.compile()
    return nc


TRACE = False          # set by test harness to capture an NTFF profile
LAST_RESULTS = None    # BassKernelResults of the most recent run
_NC_CACHE = None       # compiled program, reused across calls


def kernel(x, weights, nbr_idx, valid, fault_mask):
    global LAST_RESULTS
    from concourse.bass_utils import run_bass_kernel_spmd

    x = np.asarray(x)
    out_dtype = x.dtype

    WM, WE = _build_blocks(np.asarray(weights), np.asarray(nbr_idx), np.asarray(valid))
    WM = WM.astype(_BF16)
    WE = WE.astype(_BF16)

    # x transposed to (hw, batch), zero-padded, bf16
    xt_pad = np.zeros((PAD + GRID + PAD, BATCH), dtype=_BF16)
    xt_pad[PAD : PAD + HW] = np.ascontiguousarray(x.T).astype(_BF16)

    k128 = np.arange(128)
    in_maps = []
    for c in range(NCORES):
        j0 = c * CPC
        # main windows: global window t in [j0-2, j0+70), rows PAD + 126*t - 1 + k
        tglob = j0 - 2 + np.arange(NWIN)
        rows = (PAD + C * tglob - 1)[:, None] + k128[None, :]  # (72, 128)
        xw = np.ascontiguousarray(xt_pad[rows].transpose(1, 0, 2))  # (128, 72, 256)

        # merged edge windows: rows 0:4 from [126J+379, +4), 4:8 from [126J-257, +4)
        Jc = j0 + np.arange(CPC)
        k4 = np.arange(4)
        erows_p = (PAD + C * Jc + 379)[:, None] + k4[None, :]  # (66, 4)
        erows_m = (PAD + C * Jc - 257)[:, None] + k4[None, :]  # (66, 4)
        xc = np.concatenate(
            [xt_pad[erows_p], xt_pad[erows_m]], axis=1
        )  # (66, 8, 256)
        xce = xc[0::2].reshape(NSLAB, SLAB // 2, 8, 256).transpose(0, 2, 1, 3)
        xco = xc[1::2].reshape(NSLAB, SLAB // 2, 8, 256).transpose(0, 2, 1, 3)

        wm_c = (
            WM[j0 : j0 + CPC]
            .reshape(NSLAB, SLAB, 128, 384)
            .transpose(0, 2, 1, 3)
        )  # (11, 128, 6, 384)
        we_cc = WE[j0 : j0 + CPC]
        wee = we_cc[0::2].reshape(NSLAB, SLAB // 2, 8, 128).transpose(0, 2, 1, 3)
        weo = we_cc[1::2].reshape(NSLAB, SLAB // 2, 8, 128).transpose(0, 2, 1, 3)

        in_maps.append(
            {
                "xw": xw.reshape(128, NWIN * 256),
                "wm": np.ascontiguousarray(wm_c).reshape(NSLAB, 128, SLAB * 384),
                "xce": np.ascontiguousarray(xce).reshape(NSLAB, 8, (SLAB // 2) * 256),
                "xco": np.ascontiguousarray(xco).reshape(NSLAB, 8, (SLAB // 2) * 256),
                "wee": np.ascontiguousarray(wee).reshape(NSLAB, 8, (SLAB // 2) * 128),
                "weo": np.ascontiguousarray(weo).reshape(NSLAB, 8, (SLAB // 2) * 128),
            }
        )

    global _NC_CACHE
    if _NC_CACHE is None:
        _NC_CACHE = _build_program()
    nc = _NC_CACHE
    res = run_bass_kernel_spmd(
        nc, in_maps, core_ids=list(range(NCORES)), trace=TRACE
    )
    LAST_RESULTS = res

    # unshard: per-core yt is (126, 66*256) partition-major -> (B, HW)
    yts = [
        r["yt"].reshape(C, CPC, BATCH).transpose(1, 0, 2).reshape(CPC * C, BATCH)
        for r in res.results
    ]
    yt = np.concatenate(yts, axis=0)  # (66528, 256)
    y = np.ascontiguousarray(yt[:HW].T).astype(out_dtype, copy=False)
    # faulted units: reference computes sigmoid(where(fault, y, 0)) -> 0.5
    fault = np.asarray(fault_mask).astype(bool)
    y[:, ~fault] = np.float32(0.5)
    return y

